# revision 3
# baseline (speedup 1.0000x reference)
"""MiMo-V2 MoE gate routing kernel for 8 Trainium2 NeuronCores.

Problem: hidden_states [4,4096,4096] f32 -> gating GEMM vs 256 experts ->
sigmoid -> grouped top-k routing (8 groups, group score = sum of top-2,
keep top-4 groups, top-8 experts overall) -> normalized weights * 2.5.

Sharding: token-parallel, 2048 tokens/core, weights replicated, no comms.

GEMM scheme ("B"): precision-split so the 4096-deep contraction costs
~2.1 PE cycles/row instead of fp32's 4 or the 3-pass fp32r split's 3:

    logits*2^27 =  x @ (wh*2^27)              fp32r pass (PE rounds x to
                                              rne12(x) internally)
                + e4m3(x*2^5)    (x) e4m3(wl*2^22)   \  one fp8 DoubleRow
                + e4m3(xl*2^17)  (x) e4m3(wh*2^10)   /  pass (slot-paired)

where wh = rne12(W), wl = W - wh, xl = x - rne12(x) (xl8 precomputed on
host; rne12 == the PE's fp32r operand rounding, probe-verified). Both fp8
correction terms land at scale 2^27, so main + corrections accumulate in
ONE PSUM bank and the 2^-27 descale folds into the sigmoid's scale.
Simulated accuracy vs fp32 reference: 2/131072 idx mismatches, rel 1.6e-3.

Orientation: weights are the matmul stationary ([128 hidden x 128 expert]
chunks), tokens stream 512 wide, so LDWEIGHTS (~107ns) hides under 213ns+
streams. GEMM output is [expert, token]; scores are sigmoid'd on ScalarE
(PSUM->SBUF), transposed 128x128 by the PE, and routed with DVE sort ops.

Device layout (per core):
  xt   [128, 32, 2048] f32r  xt[p,kc,t] = X[t, kc*128+p] (raw fp32 bits)
  xl8t [128, 32, 2048] e4m3  e4m3((X - rne12(X))*2^17), same layout
  wt   [128, 32, 2, 128] f32r (rne12(W)*2^27)[eh*128+e, kc*128+p]
  wt8  [128, 32, 2, 2, 128] e4m3  s=0: e4m3(wl*2^22), s=1: e4m3(wh*2^10)
  bias [128, 2] f32          bias[eh*128+p]
  idn  [128, 128] f32        identity (PE transpose)
  oidx [128, 16, 8] i32      oidx[t,tt,k], token = tt*128 + t
  ow   [128, 16, 8] f32
"""

from contextlib import ExitStack

import numpy as np
import ml_dtypes

import concourse.bacc as bacc
import concourse.mybir as mybir
import concourse.tile as tile
from concourse.bass_utils import run_bass_kernel_spmd

P = 128
H = 4096
E = 256
KC = H // P          # 32 hidden chunks
NCORES = 8
T = 16384
TPC = T // NCORES    # 2048 tokens per core
CHUNK = 512
NCH = TPC // CHUNK   # 4 chunks per core
KQ = 4               # kc per x tile (DMA batch)
NQ = KC // KQ        # 8 x tiles per chunk
NT = TPC // P        # 16 output token tiles
N_GROUP = 8
TOPK_GROUP = 4
TOP_K = 8
ROUTED_SCALE = 2.5
NEG_BIG = 1.0e30

SC_X8 = 5            # xh8 = e4m3(x * 2^5)
SC_WL = 22           # wl8 = e4m3(wl * 2^22)
SC_XL = 17           # xl8 = e4m3(xl * 2^17)
SC_WH = 10           # wh8 = e4m3(wh * 2^10)
SC_PSUM = 27         # main/corr accumulate at 2^27

TRACE = False
E4NP = ml_dtypes.float8_e4m3

_CACHE = {}


def _build(reps=1):
    f32 = mybir.dt.float32
    f32r = mybir.dt.float32r
    e4 = mybir.dt.float8e4
    nc = bacc.Bacc(
        "TRN2", target_bir_lowering=False, debug=False, enable_asserts=False
    )
    xt = nc.dram_tensor("xt", [P, KC, TPC], f32r, kind="ExternalInput").ap()
    xl8t = nc.dram_tensor("xl8t", [P, KC, TPC], e4, kind="ExternalInput").ap()
    wt = nc.dram_tensor("wt", [P, KC, 2, P], f32r, kind="ExternalInput").ap()
    wt8 = nc.dram_tensor("wt8", [P, KC, 2, 2, P], e4, kind="ExternalInput").ap()
    bias = nc.dram_tensor("bias", [P, 2], f32, kind="ExternalInput").ap()
    idn = nc.dram_tensor("idn", [P, P], f32, kind="ExternalInput").ap()
    oidx = nc.dram_tensor("oidx", [P, NT, TOP_K], mybir.dt.int32,
                          kind="ExternalOutput").ap()
    ow = nc.dram_tensor("ow", [P, NT, TOP_K], f32, kind="ExternalOutput").ap()

    with tile.TileContext(nc) as tc, ExitStack() as ctx:
        if reps == 1:
            _body(ctx, tc, xt, xl8t, wt, wt8, bias, idn, oidx, ow)
        else:
            with tc.For_i(0, reps, 1):
                with ExitStack() as ictx:
                    _body(ictx, tc, xt, xl8t, wt, wt8, bias, idn, oidx, ow)
    nc.compile()
    return nc


def _body(ctx, tc, xt, xl8t, wt, wt8, bias, idn, oidx, ow):
    nc = tc.nc
    f32 = mybir.dt.float32
    e4 = mybir.dt.float8e4
    Alu = mybir.AluOpType

    wpool = ctx.enter_context(tc.tile_pool(name="wpool", bufs=1))
    xpool = ctx.enter_context(tc.tile_pool(name="xpool", bufs=NQ + 1))
    x8pool = ctx.enter_context(tc.tile_pool(name="x8pool", bufs=NQ + 1))
    scpool = ctx.enter_context(tc.tile_pool(name="scpool", bufs=2))
    stpool = ctx.enter_context(tc.tile_pool(name="stpool", bufs=3))
    gpool = ctx.enter_context(tc.tile_pool(name="gpool", bufs=3))
    apool = ctx.enter_context(tc.tile_pool(name="apool", bufs=1))
    psa = ctx.enter_context(tc.tile_pool(name="psa", bufs=4, space="PSUM"))
    pst = ctx.enter_context(tc.tile_pool(name="pst", bufs=2, space="PSUM"))

    wsb = wpool.tile([P, KC, 2, P], wt.dtype)
    for ws in range(4):
        lo, hi = ws * KC // 4, (ws + 1) * KC // 4
        nc.sync.dma_start(wsb[:, lo:hi], wt[:, lo:hi])
    w8sb = wpool.tile([P, KC, 2, 2, P], e4)
    nc.sync.dma_start(w8sb[:], wt8)
    bsb = wpool.tile([P, 2], f32)
    nc.sync.dma_start(bsb[:], bias)
    isb = wpool.tile([P, P], f32)
    nc.sync.dma_start(isb[:], idn)
    oi_acc = apool.tile([P, NT, TOP_K], mybir.dt.int32)
    owt_acc = apool.tile([P, NT, TOP_K], f32)

    def gemm(ch):
        t0 = ch * CHUNK
        xs, x8 = [], []
        for q in range(NQ):
            k0 = q * KQ
            xq = xpool.tile([P, KQ, CHUNK], xt.dtype, tag="xq")
            nc.sync.dma_start(xq[:], xt[:, k0:k0 + KQ, t0:t0 + CHUNK])
            x8q = x8pool.tile([P, KQ, 2, CHUNK], e4, tag="x8q")
            nc.sync.dma_start(
                x8q[:, :, 1], xl8t[:, k0:k0 + KQ, t0:t0 + CHUNK]
            )
            # xh8 = e4m3(x * 2^5); alternate DVE/ACT to balance load
            if q % 2 == 0:
                nc.vector.tensor_scalar(
                    x8q[:, :, 0], xq[:], float(2.0**SC_X8), None, Alu.mult
                )
            else:
                nc.scalar.mul(x8q[:, :, 0], xq[:], float(2.0**SC_X8))
            xs.append(xq)
            x8.append(x8q)

        pa = [psa.tile([P, CHUNK], f32, tag="pa", name=f"pa{ch}_{eh}")
              for eh in range(2)]
        for kc in range(KC):
            for eh in range(2):
                nc.tensor.matmul(
                    pa[eh][:], lhsT=wsb[:, kc, eh], rhs=xs[kc // KQ][:, kc % KQ],
                    start=(kc == 0), stop=False,
                )
        for kc in range(KC):
            for eh in range(2):
                nc.tensor.matmul(
                    pa[eh][:], lhsT=w8sb[:, kc, :, eh],
                    rhs=x8[kc // KQ][:, kc % KQ],
                    perf_mode=mybir.MatmulPerfMode.DoubleRow,
                    start=False, stop=(kc == KC - 1),
                )
        return pa

    def post(ch, pa):
        # scores = sigmoid(psum * 2^-27) + bias (bias==0 in this problem)
        sc = scpool.tile([P, 2, CHUNK], f32, tag="sc")
        for eh in range(2):
            nc.scalar.activation(
                sc[:, eh], pa[eh][:], mybir.ActivationFunctionType.Sigmoid,
                scale=float(2.0**-SC_PSUM),
            )
            nc.vector.tensor_scalar(
                sc[:, eh], sc[:, eh], bsb[:, eh:eh + 1], None, Alu.add
            )
        for tg in range(CHUNK // P):
            tt = ch * (CHUNK // P) + tg
            pt = pst.tile([P, E], f32, tag="pt")
            for eh in range(2):
                nc.tensor.transpose(
                    pt[:, eh * P:(eh + 1) * P],
                    sc[:, eh, tg * P:(tg + 1) * P], isb[:],
                )
            sct = stpool.tile([P, E], f32, tag="sct")
            nc.vector.tensor_copy(sct[:], pt[:])
            route(tt, sct)

    def route(tt, sct):
        sc3 = sct[:].rearrange("p (g k) -> p g k", g=N_GROUP)
        # group scores: sum of top-2 within each group of 32
        gt = gpool.tile([P, N_GROUP, 8], f32, tag="gt")
        for g in range(N_GROUP):
            nc.vector.max(gt[:, g], sc3[:, g])
        gs = gpool.tile([P, N_GROUP], f32, tag="gs")
        nc.vector.tensor_tensor(gs[:], gt[:, :, 0], gt[:, :, 1], Alu.add)
        # top-4 groups: mask = gs >= (4th largest group score)
        gm = gpool.tile([P, 8], f32, tag="gm")
        nc.vector.max(gm[:], gs[:])
        mk = gpool.tile([P, N_GROUP], f32, tag="mk")
        nc.vector.tensor_scalar(
            mk[:], gs[:], gm[:, TOPK_GROUP - 1:TOPK_GROUP], None, Alu.is_ge
        )
        # mk -> 0 for selected groups, -1e30 for unselected
        nc.vector.tensor_scalar(mk[:], mk[:], 1.0, NEG_BIG, Alu.subtract, Alu.mult)
        tmp = stpool.tile([P, E], f32, tag="tmp")
        tmp3 = tmp[:].rearrange("p (g k) -> p g k", g=N_GROUP)
        for g in range(N_GROUP):
            nc.vector.tensor_scalar(
                tmp3[:, g], sc3[:, g], mk[:, g:g + 1], None, Alu.add
            )
        # top-8 experts (HW sort unit); ties resolve to lowest index like jax
        v8 = gpool.tile([P, TOP_K], f32, tag="v8")
        nc.vector.max(v8[:], tmp[:])
        i8 = gpool.tile([P, TOP_K], mybir.dt.uint32, tag="i8")
        nc.vector.max_index(i8[:], v8[:], tmp[:])
        # normalize: w = v8 * (2.5 / (sum(v8) + 1e-20))
        den = gpool.tile([P, 1], f32, tag="den")
        nc.vector.tensor_reduce(den[:], v8[:], axis=mybir.AxisListType.X, op=Alu.add)
        nc.vector.tensor_scalar_add(den[:], den[:], 1e-20)
        rec = gpool.tile([P, 1], f32, tag="rec")
        nc.vector.reciprocal(rec[:], den[:])
        nc.vector.tensor_scalar_mul(rec[:], rec[:], ROUTED_SCALE)
        nc.vector.tensor_scalar(owt_acc[:, tt], v8[:], rec[:], None, Alu.mult)
        nc.vector.tensor_copy(oi_acc[:, tt], i8[:])

    # software pipeline: GEMM(ch) runs one chunk ahead of post-processing
    pas = [None] * NCH
    pas[0] = gemm(0)
    for ch in range(1, NCH):
        pas[ch] = gemm(ch)
        post(ch - 1, pas[ch - 1])
    post(NCH - 1, pas[NCH - 1])

    nc.sync.dma_start(oidx, oi_acc[:])
    nc.sync.dma_start(ow, owt_acc[:])


def _get_nc(reps=1):
    if reps not in _CACHE:
        _CACHE[reps] = _build(reps)
    return _CACHE[reps]


def _rne12(a):
    """Round fp32 to 12 significant bits, nearest-even (= PE fp32r rounding)."""
    u = np.ascontiguousarray(a, dtype=np.float32).view(np.uint32)
    low = u & np.uint32(0xFFF)
    hi = u >> np.uint32(12)
    up = (low > 0x800) | ((low == 0x800) & ((hi & np.uint32(1)) == 1))
    return ((hi + up.astype(np.uint32)) << np.uint32(12)).view(np.float32)


def make_in_maps(hidden_states, weight, e_score_correction_bias, sim_round=False):
    x = np.ascontiguousarray(hidden_states, dtype=np.float32).reshape(T, H)
    w = np.ascontiguousarray(weight, dtype=np.float32)

    xh = _rne12(x)
    xl8 = ((x.astype(np.float64) - xh) * 2.0**SC_XL).astype(np.float32)
    xl8 = xl8.astype(E4NP)
    if sim_round:
        # CoreSim does not model fp32r rounding; pre-round x for sim parity
        x = xh

    def tok_layout(a):  # [TPC, H] -> [P, KC, TPC]
        return np.ascontiguousarray(a.reshape(TPC, KC, P).transpose(2, 1, 0))

    wh = _rne12(w)
    wl = (w.astype(np.float64) - wh).astype(np.float32)

    def w_layout(a):  # [E, H] -> [P, KC, 2, P]
        return a.reshape(2, P, KC, P).transpose(3, 2, 0, 1)

    wt = np.ascontiguousarray(w_layout(wh * 2.0**SC_PSUM).astype(np.float32))
    wl8 = w_layout((wl * 2.0**SC_WL).astype(E4NP))
    wh8 = w_layout((wh.astype(np.float64) * 2.0**SC_WH).astype(E4NP))
    wt8 = np.ascontiguousarray(np.stack([wl8, wh8], axis=2))

    bias = np.ascontiguousarray(
        np.asarray(e_score_correction_bias, dtype=np.float32).reshape(2, P).T
    )
    idn = np.eye(P, dtype=np.float32)

    maps = []
    for c in range(NCORES):
        s = slice(c * TPC, (c + 1) * TPC)
        maps.append({
            "xt": tok_layout(x[s]),
            "xl8t": tok_layout(xl8[s]),
            "wt": wt, "wt8": wt8, "bias": bias, "idn": idn,
        })
    return maps


def gather_outputs(out_maps):
    idx = np.stack([m["oidx"] for m in out_maps])   # [c, p, tt, k]
    w = np.stack([m["ow"] for m in out_maps])
    idx = idx.transpose(0, 2, 1, 3).reshape(T, TOP_K)
    w = w.transpose(0, 2, 1, 3).reshape(T, TOP_K)
    return np.ascontiguousarray(idx.astype(np.int32)), np.ascontiguousarray(w)


def kernel(hidden_states, weight, e_score_correction_bias):
    nc = _get_nc()
    in_maps = make_in_maps(hidden_states, weight, e_score_correction_bias)
    res = run_bass_kernel_spmd(
        nc, in_maps, core_ids=list(range(NCORES)), trace=TRACE
    )
    kernel.last_results = res
    return gather_outputs(res.results)


# revision 6
# speedup vs baseline: 1.1694x; 1.1694x over previous
"""MiMo-V2 MoE gate routing kernel for 8 Trainium2 NeuronCores.

Problem: hidden_states [4,4096,4096] f32 -> gating GEMM vs 256 experts ->
sigmoid -> grouped top-k routing (8 groups, group score = sum of top-2,
keep top-4 groups, top-8 experts overall) -> normalized weights * 2.5.

Sharding: token-parallel, 2048 tokens/core, weights replicated, no comms.

GEMM scheme ("B"): precision-split so the 4096-deep contraction costs
~2.1 PE cycles/row instead of fp32's 4 or the 3-pass fp32r split's 3:

    logits*2^27 =  x @ (wh*2^27)              fp32r pass (PE rounds x to
                                              rne12(x) internally)
                + e4m3(x*2^5)    (x) e4m3(wl*2^22)   \  one fp8 DoubleRow
                + e4m3(xl*2^17)  (x) e4m3(wh*2^10)   /  pass (slot-paired)

where wh = rne12(W), wl = W - wh, xl = x - rne12(x) (xl8 precomputed on
host; rne12 == the PE's fp32r operand rounding, probe-verified). Both fp8
correction terms land at scale 2^27, so main + corrections accumulate in
ONE PSUM bank and the 2^-27 descale folds into the sigmoid's scale.
Simulated accuracy vs fp32 reference: 2/131072 idx mismatches, rel 1.6e-3.

Orientation: weights are the matmul stationary ([128 hidden x 128 expert]
chunks), tokens stream 512 wide, so LDWEIGHTS (~107ns) hides under 213ns+
streams. GEMM output is [expert, token]; scores are sigmoid'd on ScalarE
(PSUM->SBUF), transposed 128x128 by the PE, and routed with DVE sort ops.

Device layout (per core):
  xt   [128, 32, 2048] f32r  xt[p,kc,t] = X[t, kc*128+p] (raw fp32 bits)
  xl8t [128, 32, 2048] e4m3  e4m3((X - rne12(X))*2^17), same layout
  wt   [128, 32, 2, 128] f32r (rne12(W)*2^27)[eh*128+e, kc*128+p]
  wt8  [128, 32, 2, 2, 128] e4m3  s=0: e4m3(wl*2^22), s=1: e4m3(wh*2^10)
  bias [128, 2] f32          bias[eh*128+p]
  idn  [128, 128] f32        identity (PE transpose)
  oidx [128, 16, 8] i32      oidx[t,tt,k], token = tt*128 + t
  ow   [128, 16, 8] f32
"""

from contextlib import ExitStack

import numpy as np
import ml_dtypes

import concourse.bacc as bacc
import concourse.mybir as mybir
import concourse.tile as tile
from concourse.bass_utils import run_bass_kernel_spmd

P = 128
H = 4096
E = 256
KC = H // P          # 32 hidden chunks
NCORES = 8
T = 16384
TPC = T // NCORES    # 2048 tokens per core
CHUNK = 512
NCH = TPC // CHUNK   # 4 chunks per core
KQ = 4               # kc per x tile (DMA batch)
NQ = KC // KQ        # 8 x tiles per chunk
NT = TPC // P        # 16 output token tiles
N_GROUP = 8
TOPK_GROUP = 4
TOP_K = 8
ROUTED_SCALE = 2.5
NEG_BIG = 1.0e30

SC_X8 = 5            # xh8 = e4m3(x * 2^5)
SC_WL = 22           # wl8 = e4m3(wl * 2^22)
SC_XL = 17           # xl8 = e4m3(xl * 2^17)
SC_WH = 10           # wh8 = e4m3(wh * 2^10)
SC_PSUM = 27         # main/corr accumulate at 2^27

TRACE = False
E4NP = ml_dtypes.float8_e4m3

# timing-experiment knobs (numerics invalid when GEMM passes are disabled)
EN_MAIN = True       # emit main fp32r pass
EN_CORR = True       # emit fp8 DoubleRow correction pass
EN_POST = True       # emit transpose + routing (else dummy outputs)

_CACHE = {}


def _build(reps=1):
    f32 = mybir.dt.float32
    f32r = mybir.dt.float32r
    e4 = mybir.dt.float8e4
    nc = bacc.Bacc(
        "TRN2", target_bir_lowering=False, debug=False, enable_asserts=False
    )
    xt = nc.dram_tensor("xt", [P, KC, TPC], f32r, kind="ExternalInput").ap()
    xl8t = nc.dram_tensor("xl8t", [P, KC, TPC], e4, kind="ExternalInput").ap()
    wt = nc.dram_tensor("wt", [P, KC, 2, P], f32r, kind="ExternalInput").ap()
    wt8 = nc.dram_tensor("wt8", [P, KC, 2, 2, P], e4, kind="ExternalInput").ap()
    bias = nc.dram_tensor("bias", [P, 2], f32, kind="ExternalInput").ap()
    idn = nc.dram_tensor("idn", [P, P], f32, kind="ExternalInput").ap()
    oidx = nc.dram_tensor("oidx", [P, NT, TOP_K], mybir.dt.int32,
                          kind="ExternalOutput").ap()
    ow = nc.dram_tensor("ow", [P, NT, TOP_K], f32, kind="ExternalOutput").ap()

    with tile.TileContext(nc) as tc, ExitStack() as ctx:
        if reps == 1:
            _body(ctx, tc, xt, xl8t, wt, wt8, bias, idn, oidx, ow)
        else:
            with tc.For_i(0, reps, 1):
                with ExitStack() as ictx:
                    _body(ictx, tc, xt, xl8t, wt, wt8, bias, idn, oidx, ow)
    nc.compile()
    return nc


def _body(ctx, tc, xt, xl8t, wt, wt8, bias, idn, oidx, ow):
    nc = tc.nc
    f32 = mybir.dt.float32
    e4 = mybir.dt.float8e4
    Alu = mybir.AluOpType

    wpool = ctx.enter_context(tc.tile_pool(name="wpool", bufs=1))
    xpool = ctx.enter_context(tc.tile_pool(name="xpool", bufs=NQ + 1))
    x8pool = ctx.enter_context(tc.tile_pool(name="x8pool", bufs=NQ + 1))
    scpool = ctx.enter_context(tc.tile_pool(name="scpool", bufs=2))
    stpool = ctx.enter_context(tc.tile_pool(name="stpool", bufs=3))
    gpool = ctx.enter_context(tc.tile_pool(name="gpool", bufs=3))
    apool = ctx.enter_context(tc.tile_pool(name="apool", bufs=1))
    psa = ctx.enter_context(tc.tile_pool(name="psa", bufs=4, space="PSUM"))
    pst = ctx.enter_context(tc.tile_pool(name="pst", bufs=2, space="PSUM"))

    wsb = wpool.tile([P, KC, 2, P], wt.dtype)
    for ws in range(4):
        lo, hi = ws * KC // 4, (ws + 1) * KC // 4
        nc.sync.dma_start(wsb[:, lo:hi], wt[:, lo:hi])
    w8sb = wpool.tile([P, KC, 2, 2, P], e4)
    nc.sync.dma_start(w8sb[:], wt8)
    bsb = wpool.tile([P, 2], f32)
    nc.sync.dma_start(bsb[:], bias)
    isb = wpool.tile([P, P], f32)
    nc.sync.dma_start(isb[:], idn)
    oi_acc = apool.tile([P, NT, TOP_K], mybir.dt.int32)
    owt_acc = apool.tile([P, NT, TOP_K], f32)

    def gemm(ch):
        t0 = ch * CHUNK
        xs, x8 = [], []
        for q in range(NQ):
            k0 = q * KQ
            xq = xpool.tile([P, KQ, CHUNK], xt.dtype, tag="xq")
            nc.sync.dma_start(xq[:], xt[:, k0:k0 + KQ, t0:t0 + CHUNK])
            xs.append(xq)
            if not EN_CORR:
                continue
            x8q = x8pool.tile([P, KQ, 2, CHUNK], e4, tag="x8q")
            nc.sync.dma_start(
                x8q[:, :, 1], xl8t[:, k0:k0 + KQ, t0:t0 + CHUNK]
            )
            # xh8 = e4m3(x * 2^5); alternate DVE/ACT to balance load
            if q % 2 == 0:
                nc.vector.tensor_scalar(
                    x8q[:, :, 0], xq[:], float(2.0**SC_X8), None, Alu.mult
                )
            else:
                nc.scalar.mul(x8q[:, :, 0], xq[:], float(2.0**SC_X8))
            x8.append(x8q)

        pa = [psa.tile([P, CHUNK], f32, tag="pa", name=f"pa{ch}_{eh}")
              for eh in range(2)]
        if EN_MAIN:
            for kc in range(KC):
                for eh in range(2):
                    nc.tensor.matmul(
                        pa[eh][:], lhsT=wsb[:, kc, eh],
                        rhs=xs[kc // KQ][:, kc % KQ],
                        start=(kc == 0), stop=(not EN_CORR and kc == KC - 1),
                    )
        if EN_CORR:
            for kc in range(KC):
                for eh in range(2):
                    nc.tensor.matmul(
                        pa[eh][:], lhsT=w8sb[:, kc, :, eh],
                        rhs=x8[kc // KQ][:, kc % KQ],
                        perf_mode=mybir.MatmulPerfMode.DoubleRow,
                        start=(not EN_MAIN and kc == 0), stop=(kc == KC - 1),
                    )
        return pa

    def post(ch, pa):
        # scores = sigmoid(psum * 2^-27) + bias (bias==0 in this problem)
        sc = scpool.tile([P, 2, CHUNK], f32, tag="sc")
        for eh in range(2):
            nc.scalar.activation(
                sc[:, eh], pa[eh][:], mybir.ActivationFunctionType.Sigmoid,
                scale=float(2.0**-SC_PSUM),
            )
            nc.vector.tensor_scalar(
                sc[:, eh], sc[:, eh], bsb[:, eh:eh + 1], None, Alu.add
            )
        for tg in range(CHUNK // P):
            tt = ch * (CHUNK // P) + tg
            pt = pst.tile([P, E], f32, tag="pt")
            for eh in range(2):
                nc.tensor.transpose(
                    pt[:, eh * P:(eh + 1) * P],
                    sc[:, eh, tg * P:(tg + 1) * P], isb[:],
                )
            sct = stpool.tile([P, E], f32, tag="sct")
            nc.vector.tensor_copy(sct[:], pt[:])
            route(tt, sct)

    def route(tt, sct):
        sc3 = sct[:].rearrange("p (g k) -> p g k", g=N_GROUP)
        # group scores: sum of top-2 within each group of 32
        gt = gpool.tile([P, N_GROUP, 8], f32, tag="gt")
        for g in range(N_GROUP):
            nc.vector.max(gt[:, g], sc3[:, g])
        gs = gpool.tile([P, N_GROUP], f32, tag="gs")
        nc.vector.tensor_tensor(gs[:], gt[:, :, 0], gt[:, :, 1], Alu.add)
        # top-4 groups: mask = gs >= (4th largest group score)
        gm = gpool.tile([P, 8], f32, tag="gm")
        nc.vector.max(gm[:], gs[:])
        mk = gpool.tile([P, N_GROUP], f32, tag="mk")
        nc.vector.tensor_scalar(
            mk[:], gs[:], gm[:, TOPK_GROUP - 1:TOPK_GROUP], None, Alu.is_ge
        )
        # mk -> 0 for selected groups, -1e30 for unselected
        nc.vector.tensor_scalar(mk[:], mk[:], 1.0, NEG_BIG, Alu.subtract, Alu.mult)
        tmp = stpool.tile([P, E], f32, tag="tmp")
        tmp3 = tmp[:].rearrange("p (g k) -> p g k", g=N_GROUP)
        for g in range(N_GROUP):
            nc.vector.tensor_scalar(
                tmp3[:, g], sc3[:, g], mk[:, g:g + 1], None, Alu.add
            )
        # top-8 experts (HW sort unit); ties resolve to lowest index like jax
        v8 = gpool.tile([P, TOP_K], f32, tag="v8")
        nc.vector.max(v8[:], tmp[:])
        i8 = gpool.tile([P, TOP_K], mybir.dt.uint32, tag="i8")
        nc.vector.max_index(i8[:], v8[:], tmp[:])
        # normalize: w = v8 * (2.5 / (sum(v8) + 1e-20))
        den = gpool.tile([P, 1], f32, tag="den")
        nc.vector.tensor_reduce(den[:], v8[:], axis=mybir.AxisListType.X, op=Alu.add)
        nc.vector.tensor_scalar_add(den[:], den[:], 1e-20)
        rec = gpool.tile([P, 1], f32, tag="rec")
        nc.vector.reciprocal(rec[:], den[:])
        nc.vector.tensor_scalar_mul(rec[:], rec[:], ROUTED_SCALE)
        nc.vector.tensor_scalar(owt_acc[:, tt], v8[:], rec[:], None, Alu.mult)
        nc.vector.tensor_copy(oi_acc[:, tt], i8[:])

    # software pipeline: GEMM(ch) runs one chunk ahead of post-processing
    pas = [None] * NCH
    pas[0] = gemm(0)
    for ch in range(1, NCH):
        pas[ch] = gemm(ch)
        if EN_POST:
            post(ch - 1, pas[ch - 1])
    if EN_POST:
        post(NCH - 1, pas[NCH - 1])
    else:
        for eh in range(2):
            nc.scalar.activation(
                owt_acc[:, eh * 8:eh * 8 + 8].rearrange("p a b -> p (a b)"),
                pas[NCH - 1][eh][:, :64],
                mybir.ActivationFunctionType.Sigmoid, scale=float(2.0**-SC_PSUM),
            )
        nc.vector.tensor_copy(oi_acc[:], owt_acc[:])

    nc.sync.dma_start(oidx, oi_acc[:])
    nc.sync.dma_start(ow, owt_acc[:])


def _get_nc(reps=1):
    if reps not in _CACHE:
        _CACHE[reps] = _build(reps)
    return _CACHE[reps]


def _rne12(a):
    """Round fp32 to 12 significant bits, nearest-even (= PE fp32r rounding)."""
    u = np.ascontiguousarray(a, dtype=np.float32).view(np.uint32)
    low = u & np.uint32(0xFFF)
    hi = u >> np.uint32(12)
    up = (low > 0x800) | ((low == 0x800) & ((hi & np.uint32(1)) == 1))
    return ((hi + up.astype(np.uint32)) << np.uint32(12)).view(np.float32)


def make_in_maps(hidden_states, weight, e_score_correction_bias, sim_round=False):
    x = np.ascontiguousarray(hidden_states, dtype=np.float32).reshape(T, H)
    w = np.ascontiguousarray(weight, dtype=np.float32)

    xh = _rne12(x)
    xl8 = ((x.astype(np.float64) - xh) * 2.0**SC_XL).astype(np.float32)
    xl8 = xl8.astype(E4NP)
    if sim_round:
        # CoreSim does not model fp32r rounding; pre-round x for sim parity
        x = xh

    def tok_layout(a):  # [TPC, H] -> [P, KC, TPC]
        return np.ascontiguousarray(a.reshape(TPC, KC, P).transpose(2, 1, 0))

    wh = _rne12(w)
    wl = (w.astype(np.float64) - wh).astype(np.float32)

    def w_layout(a):  # [E, H] -> [P, KC, 2, P]
        return a.reshape(2, P, KC, P).transpose(3, 2, 0, 1)

    wt = np.ascontiguousarray(w_layout(wh * 2.0**SC_PSUM).astype(np.float32))
    wl8 = w_layout((wl * 2.0**SC_WL).astype(E4NP))
    wh8 = w_layout((wh.astype(np.float64) * 2.0**SC_WH).astype(E4NP))
    wt8 = np.ascontiguousarray(np.stack([wl8, wh8], axis=2))

    bias = np.ascontiguousarray(
        np.asarray(e_score_correction_bias, dtype=np.float32).reshape(2, P).T
    )
    idn = np.eye(P, dtype=np.float32)

    maps = []
    for c in range(NCORES):
        s = slice(c * TPC, (c + 1) * TPC)
        maps.append({
            "xt": tok_layout(x[s]),
            "xl8t": tok_layout(xl8[s]),
            "wt": wt, "wt8": wt8, "bias": bias, "idn": idn,
        })
    return maps


def gather_outputs(out_maps):
    idx = np.stack([m["oidx"] for m in out_maps])   # [c, p, tt, k]
    w = np.stack([m["ow"] for m in out_maps])
    idx = idx.transpose(0, 2, 1, 3).reshape(T, TOP_K)
    w = w.transpose(0, 2, 1, 3).reshape(T, TOP_K)
    return np.ascontiguousarray(idx.astype(np.int32)), np.ascontiguousarray(w)


def kernel(hidden_states, weight, e_score_correction_bias):
    nc = _get_nc()
    in_maps = make_in_maps(hidden_states, weight, e_score_correction_bias)
    res = run_bass_kernel_spmd(
        nc, in_maps, core_ids=list(range(NCORES)), trace=TRACE
    )
    kernel.last_results = res
    return gather_outputs(res.results)
